# revision 16
# baseline (speedup 1.0000x reference)
"""Trainium2 Bass kernel for CapLayer2 (1x1-conv capsule layer with dynamic routing).

Sharding: data-parallel over batch — 8 batches per core on 8 NeuronCores.

Per-core design (diagonal wavefront over 8 batches):
  - Emission interleaves conv(b), it0(b-1), it1(b-2), it2(b-3) so the
    routing chains (softmax -> s -> squash -> V -> delta) hide inside the
    PE-saturated conv stream and every engine queue stays deep.
  - The 1x1 conv produces BOTH pred layouts on TensorE:
      predT [i-part, o] f32r  (for the s matmuls, contraction over i=1024)
      pred  [o-part, i] bf16  (stationary operand of the delta matmuls)
    with predT/pred tiles interleaved so their DVE/ACT bias evictions run
    concurrently instead of in phases.
  - delta is computed TRANSPOSED: matmul(lhsT=pred[o,i-chunk], rhs=V[o,j])
    gives [i-part, j] tiles straight from PE (no [10,1024] delta streams,
    no delta transposes); a single [128,80] DVE op folds them into b.
  - s/squash per batch use [10, 320] tiles; norms via ACT Square+accum;
    sqrt as exp(0.5*ln) with the ACT table pinned so it never reloads.
  - The otherwise-idle GPSIMD (Pool) takes the squash scalar chain and the
    v_full scaling; the final v is reduced to compact [10, 32] per batch
    and written with one gathered DMA for all 8 batches.
"""

import numpy as np
from contextlib import ExitStack

import concourse.bacc as bacc
import concourse.bass as bass
import concourse.hw_specs as hw_specs

# Force every activation onto the one table that contains all functions this
# kernel uses (Copy/Identity/Exp/Ln/Square) so the ACT engine loads its
# function table exactly once instead of thrashing between sets.
_ONE_TABLE = "natural_log_exp_and_others"
_orig_get_tables = hw_specs.get_activation_tables


def _pinned_tables(arch):
    tabs = _orig_get_tables(arch)
    return {k: (v if k == _ONE_TABLE else set()) for k, v in tabs.items()}


bacc.get_activation_tables = _pinned_tables
import concourse.tile as tile
from concourse import mybir
from concourse.bass_utils import run_bass_kernel_spmd

F32 = mybir.dt.float32
F32R = mybir.dt.float32r
BF16 = mybir.dt.bfloat16
AF = mybir.ActivationFunctionType
OP = mybir.AluOpType
AX = mybir.AxisListType

N_CORES = 8
BS = 64
C_IN = 256
J = 10
D = 32
O = J * D          # 320
I = 1024           # 32*32 pixels
ROUTE_NUM = 3
B = BS // N_CORES  # 8 batches per core
N_IT = I // 128    # 8
N_KT = C_IN // 128 # 2
N_OT = 3           # o tiles: 128, 128, 64


def r(ap):
    return ap.bitcast(F32R)


def build_kernel():
    nc = bacc.Bacc("TRN2", target_bir_lowering=False, debug=False, num_devices=1)

    x_d = nc.dram_tensor("x", [B, C_IN, I], F32R, kind="ExternalInput")
    wt_d = nc.dram_tensor("wt", [C_IN, O], F32R, kind="ExternalInput")   # W.T
    wb_d = nc.dram_tensor("wb", [1, O], F32R, kind="ExternalInput")
    out_d = nc.dram_tensor("v", [B, J, D], F32, kind="ExternalOutput")

    ident_d = nc.inline_tensor(np.eye(16, dtype=np.float32), name="ident")
    bm = np.zeros((16, O), dtype=np.float32)
    for j in range(J):
        bm[j, D * j:D * j + D] = 1.0
    bmask_d = nc.inline_tensor(bm, name="bmask")
    c0_d = nc.inline_tensor(np.full((128, J), 1.0 / J, dtype=np.float32), name="c0")

    with tile.TileContext(nc) as tc:
        with ExitStack() as ctx:
            consts = ctx.enter_context(tc.tile_pool(name="consts", bufs=1))
            xpool = ctx.enter_context(tc.tile_pool(name="xp", bufs=4))
            ppT = ctx.enter_context(tc.tile_pool(name="ppT", bufs=B))
            ppO = ctx.enter_context(tc.tile_pool(name="ppO", bufs=B))
            st = ctx.enter_context(tc.tile_pool(name="st", bufs=6))
            cpool = ctx.enter_context(tc.tile_pool(name="cp", bufs=4))
            vpool = ctx.enter_context(tc.tile_pool(name="vp", bufs=4))


            # ---- constants (wt first: it gates the first conv matmul) ----
            wt_sb = consts.tile([128, N_KT * O], F32R)
            nc.sync.dma_start(
                out=wt_sb.rearrange("p (k o) -> p k o", o=O),
                in_=wt_d.ap().rearrange("(k p) o -> p k o", p=128),
            )
            C = {}
            globals_ps = {}

            def load_consts():
                # Emitted after batch 0's x DMA so these small transfers don't
                # delay the startup-critical x.
                bias_b128 = consts.tile([128, O], F32)
                wb_bc = bass.AP(
                    tensor=wb_d, offset=0, ap=[[0, 128], [1, O]]
                ).bitcast(F32)
                nc.sync.dma_start(out=bias_b128, in_=wb_bc)
                bias_col = consts.tile([128, N_OT], F32)
                for m in range(N_OT):
                    mw = 128 if m < 2 else 64
                    nc.sync.dma_start(
                        out=bias_col[0:mw, m:m + 1],
                        in_=wb_d.ap().bitcast(F32)[0:1, 128 * m:128 * m + mw],
                    )
                ident_sb = consts.tile([128, 16], F32)
                nc.sync.dma_start(out=ident_sb[:16], in_=ident_d.ap())
                bmask_sb = consts.tile([128, O], F32)
                nc.sync.dma_start(out=bmask_sb[:16], in_=bmask_d.ap())
                c0_sb = consts.tile([128, J], F32R)
                nc.sync.dma_start(out=c0_sb, in_=r(c0_d.ap()))
                ones_sb = consts.tile([128, N_IT], F32)
                nc.gpsimd.memset(ones_sb, 1.0)
                b_sb = consts.tile([128, B * N_IT * J], F32)
                vout = consts.tile([128, B * D], F32)
                C.update(
                    bias_b128=bias_b128, bias_col=bias_col, ident_sb=ident_sb,
                    bmask_sb=bmask_sb, c0_sb=c0_sb, b_sb=b_sb, vout=vout, ones=ones_sb,
                )

            predT = [None] * B
            pred = [None] * B

            def bslice(b):
                off = b * N_IT * J
                return C["b_sb"][:, off:off + N_IT * J]

            def conv_unit(b):
                x_sb = xpool.tile([128, N_KT * I], F32R, tag="x")
                chunks = [(0, 128), (128, 512), (512, 1024)] if b == 0 else [(0, 512), (512, 1024)]
                for lo, hi in chunks:
                    for k in range(N_KT):
                        nc.sync.dma_start(
                            out=x_sb[:, k * I + lo:k * I + hi],
                            in_=x_d.ap()[b][k * 128:(k + 1) * 128, lo:hi],
                        )
                if b == 0:
                    load_consts()
                pT = ppT.tile([128, N_IT * O], F32R, tag="predT")
                pr = ppO.tile([128, N_OT * I], BF16, tag="pred")
                # Interleave the two layouts so the DVE (predT) and ACT (pred)
                # evictions run concurrently instead of in phases.
                jobs = []
                for u in range(N_IT):
                    jobs.append(("T", u))
                    if u < 2 * N_OT:
                        jobs.append(("O", u))
                for kind, u in jobs:
                    ps = globals_ps["psc"].tile([128, 512], F32, tag="cv")
                    if kind == "T":
                        for k in range(N_KT):
                            nc.tensor.matmul(
                                ps[:, :O],
                                r(x_sb[:, k * I + u * 128:k * I + u * 128 + 128]),
                                wt_sb[:, k * O:(k + 1) * O],
                                start=(k == 0),
                                stop=(k == N_KT - 1),
                            )
                        # eviction fused with the conv-bias add
                        nc.vector.tensor_tensor(
                            pT[:, u * O:(u + 1) * O], ps[:, :O], C["bias_b128"], OP.add
                        )
                    else:
                        m, h = divmod(u, 2)
                        mw = 128 if m < 2 else 64
                        for k in range(N_KT):
                            nc.tensor.matmul(
                                ps[:mw],
                                wt_sb[:, k * O + m * 128:k * O + m * 128 + mw],
                                r(x_sb[:, k * I + h * 512:k * I + (h + 1) * 512]),
                                start=(k == 0),
                                stop=(k == N_KT - 1),
                            )
                        nc.scalar.activation(
                            pr[:mw, m * I + h * 512:m * I + (h + 1) * 512],
                            ps[:mw], AF.Identity,
                            bias=C["bias_col"][0:mw, m:m + 1], scale=1.0,
                        )
                predT[b] = pT
                pred[b] = pr

            def front_unit(b, it):
                last = it == ROUTE_NUM - 1
                # ---- softmax over j (free-dim groups of 10) ----
                if it > 0:
                    e_sb = st.tile([128, N_IT * J], F32, tag="e")
                    nc.scalar.activation(e_sb, bslice(b), AF.Exp)
                    den = st.tile([128, N_IT], F32, tag="den")
                    nc.vector.reduce_sum(
                        den, e_sb.rearrange("p (g j) -> p g j", j=J), axis=AX.X
                    )
                    rden = st.tile([128, N_IT], F32, tag="rden")
                    nc.gpsimd.tensor_tensor(rden, C["ones"], den, OP.divide)
                    c_sb = cpool.tile([128, N_IT * J], F32R, tag="c")
                    nc.gpsimd.tensor_tensor(
                        c_sb.rearrange("p (g j) -> p g j", j=J),
                        e_sb.rearrange("p (g j) -> p g j", j=J),
                        rden.broadcast_to([128, N_IT, J]),
                        OP.mult,
                    )
                # ---- s = c . predT (contraction over i) ----
                ps_s = globals_ps["pss"].tile([128, O], F32, tag="s")
                for t in range(N_IT):
                    lhs = C["c0_sb"] if it == 0 else c_sb[:, t * J:(t + 1) * J]
                    nc.tensor.matmul(
                        ps_s[:J], lhs, predT[b][:, t * O:(t + 1) * O],
                        start=(t == 0), stop=(t == N_IT - 1),
                    )
                return ps_s

            def back_unit(b, it, ps_s):
                last = it == ROUTE_NUM - 1
                # ---- squash ----
                s_m = st.tile([128, O], F32, tag="s_m")
                nc.vector.tensor_tensor(s_m[:J], ps_s[:J], C["bmask_sb"][:J], OP.mult)
                ns = st.tile([128, 1], F32, tag="ns")
                if not last:
                    sq = st.tile([128, O], F32, tag="sq")
                    nc.scalar.activation(sq[:J], s_m[:J], AF.Square, accum_out=ns[:J])
                else:
                    s_cmp = st.tile([128, D], F32, tag="s_cmp")
                    nc.vector.reduce_sum(
                        s_cmp[:J],
                        s_m[:J].rearrange("p (j d) -> p d j", j=J),
                        axis=AX.X,
                    )
                    sq = st.tile([128, D], F32, tag="sqc")
                    nc.scalar.activation(sq[:J], s_cmp[:J], AF.Square, accum_out=ns[:J])
                # sqrt(ns) = exp(0.5*ln(ns)) — stays in the pinned ACT table
                lns = st.tile([128, 1], F32, tag="lns")
                nc.scalar.activation(lns[:J], ns[:J], AF.Ln)
                rt = st.tile([128, 1], F32, tag="rt")
                nc.scalar.activation(rt[:J], lns[:J], AF.Exp, scale=0.5)
                ns1 = st.tile([128, 1], F32, tag="ns1")
                nc.gpsimd.tensor_scalar_add(ns1[:J], ns[:J], 1.0)

                if last:
                    # vout = (s_cmp * sqrt(ns)) / (1 + ns) fused in one op
                    nc.vector.scalar_tensor_tensor(
                        C["vout"][:J, b * D:(b + 1) * D],
                        s_cmp[:J], rt[:J], ns1[:J].broadcast_to([J, D]),
                        OP.mult, OP.divide,
                    )
                    return
                rns1 = st.tile([128, 1], F32, tag="rns1")
                nc.vector.reciprocal(rns1[:J], ns1[:J])
                coeff = st.tile([128, 1], F32, tag="coeff")
                nc.gpsimd.tensor_tensor(coeff[:J], rt[:J], rns1[:J], OP.mult)

                v_full = st.tile([128, O], F32, tag="v_full")
                nc.gpsimd.tensor_scalar_mul(v_full[:J], s_m[:J], coeff[:J])

                # V: transpose v into block-diagonal [o-part, j]
                ps_tv = globals_ps["pst"].tile([128, 32], F32, tag="tv")
                for k in range(N_OT):
                    kw = 128 if k < 2 else 64
                    nc.tensor.transpose(
                        ps_tv[:kw, k * J:(k + 1) * J],
                        v_full[:J, k * 128:k * 128 + kw],
                        C["ident_sb"][:J, :J],
                    )
                vb = vpool.tile([128, 32], BF16, tag="vb")
                nc.scalar.activation(vb[:, :2 * J], ps_tv[:, :2 * J], AF.Identity, scale=1.0)
                nc.scalar.activation(vb[:64, 2 * J:3 * J], ps_tv[:64, 2 * J:3 * J], AF.Identity, scale=1.0)

                # delta^T: [i-part, j] tiles straight from PE, no transposes
                d_ps = globals_ps["psb"].tile([128, N_IT * J], F32, tag="d")
                for t in range(N_IT):
                    for k in range(N_OT):
                        kw = 128 if k < 2 else 64
                        nc.tensor.matmul(
                            d_ps[:, t * J:(t + 1) * J],
                            pred[b][:kw, k * I + t * 128:k * I + t * 128 + 128],
                            vb[:kw, k * J:(k + 1) * J],
                            start=(k == 0),
                            stop=(k == N_OT - 1),
                        )
                dst = bslice(b)
                if it == 0:
                    nc.vector.tensor_copy(dst, d_ps)
                else:
                    nc.vector.tensor_tensor(dst, d_ps, dst, OP.add)

            with tc.tile_pool(name="psc", bufs=4, space="PSUM") as ps_conv_:
                globals_ps["psc"] = ps_conv_
                for b in range(B):
                    conv_unit(b)
            psb = ctx.enter_context(tc.tile_pool(name="psb", bufs=2, space="PSUM"))
            pss = ctx.enter_context(tc.tile_pool(name="pss", bufs=4, space="PSUM"))
            pst = ctx.enter_context(tc.tile_pool(name="pst", bufs=2, space="PSUM"))
            globals_ps["psb"], globals_ps["pss"], globals_ps["pst"] = psb, pss, pst

            PIPE = 6
            for it in range(ROUTE_NUM):
                fr = {}
                for b in range(min(PIPE, B)):
                    fr[b] = front_unit(b, it)
                for b in range(B):
                    back_unit(b, it, fr.pop(b))
                    if b + PIPE < B:
                        fr[b + PIPE] = front_unit(b + PIPE, it)
                    if it == ROUTE_NUM - 1 and b in (3, B - 1):
                        h = 0 if b == 3 else 1
                        nc.sync.dma_start(
                            out=out_d.ap()[h * 4:(h + 1) * 4].rearrange("b j d -> j b d"),
                            in_=C["vout"][:J, h * 4 * D:(h + 1) * 4 * D]
                                .rearrange("p (b d) -> p b d", d=D),
                        )

    nc.compile()
    return nc


_NC_CACHE = None
LAST_RESULT = None


def kernel(x: np.ndarray, W: np.ndarray, W_b: np.ndarray) -> np.ndarray:
    global _NC_CACHE
    if _NC_CACHE is None:
        _NC_CACHE = build_kernel()
    nc = _NC_CACHE

    x = np.ascontiguousarray(x.reshape(BS, C_IN, I), dtype=np.float32)
    wt = np.ascontiguousarray(W.T, dtype=np.float32)
    wb = np.ascontiguousarray(W_b.reshape(1, O), dtype=np.float32)

    in_maps = [
        {
            "x": np.ascontiguousarray(x[c * B:(c + 1) * B]),
            "wt": wt,
            "wb": wb,
        }
        for c in range(N_CORES)
    ]
    import os
    trace = bool(int(os.environ.get("KERNEL_TRACE", "0")))
    res = run_bass_kernel_spmd(
        nc, in_maps, core_ids=list(range(N_CORES)), trace=trace
    )
    if trace:
        global LAST_RESULT
        LAST_RESULT = res
    out = np.concatenate([res.results[c]["v"] for c in range(N_CORES)], axis=0)
    return out.astype(np.float32)


if __name__ == "__main__":
    rng = np.random.default_rng(0)
    x = rng.standard_normal((BS, C_IN, 32, 32), dtype=np.float32)
    W = (rng.standard_normal((O, C_IN)) * 0.02).astype(np.float32)
    W_b = (rng.standard_normal((O,)) * 0.02).astype(np.float32)
    v = kernel(x=x, W=W, W_b=W_b)
    print(v.shape, v.dtype, float(np.abs(v).max()))


# revision 17
# speedup vs baseline: 1.0105x; 1.0105x over previous
"""Trainium2 Bass kernel for CapLayer2 (1x1-conv capsule layer with dynamic routing).

Sharding: data-parallel over batch — 8 batches per core on 8 NeuronCores.

Per-core design (diagonal wavefront over 8 batches):
  - Emission interleaves conv(b), it0(b-1), it1(b-2), it2(b-3) so the
    routing chains (softmax -> s -> squash -> V -> delta) hide inside the
    PE-saturated conv stream and every engine queue stays deep.
  - The 1x1 conv produces BOTH pred layouts on TensorE:
      predT [i-part, o] f32r  (for the s matmuls, contraction over i=1024)
      pred  [o-part, i] bf16  (stationary operand of the delta matmuls)
    with predT/pred tiles interleaved so their DVE/ACT bias evictions run
    concurrently instead of in phases.
  - delta is computed TRANSPOSED: matmul(lhsT=pred[o,i-chunk], rhs=V[o,j])
    gives [i-part, j] tiles straight from PE (no [10,1024] delta streams,
    no delta transposes); a single [128,80] DVE op folds them into b.
  - s/squash per batch use [10, 320] tiles; norms via ACT Square+accum;
    sqrt as exp(0.5*ln) with the ACT table pinned so it never reloads.
  - The otherwise-idle GPSIMD (Pool) takes the squash scalar chain and the
    v_full scaling; the final v is reduced to compact [10, 32] per batch
    and written with one gathered DMA for all 8 batches.
"""

import numpy as np
from contextlib import ExitStack

import concourse.bacc as bacc
import concourse.bass as bass
import concourse.hw_specs as hw_specs

# Force every activation onto the one table that contains all functions this
# kernel uses (Copy/Identity/Exp/Ln/Square) so the ACT engine loads its
# function table exactly once instead of thrashing between sets.
_ONE_TABLE = "natural_log_exp_and_others"
_orig_get_tables = hw_specs.get_activation_tables


def _pinned_tables(arch):
    tabs = _orig_get_tables(arch)
    return {k: (v if k == _ONE_TABLE else set()) for k, v in tabs.items()}


bacc.get_activation_tables = _pinned_tables
import concourse.tile as tile
from concourse import mybir
from concourse.bass_utils import run_bass_kernel_spmd

F32 = mybir.dt.float32
F32R = mybir.dt.float32r
BF16 = mybir.dt.bfloat16
AF = mybir.ActivationFunctionType
OP = mybir.AluOpType
AX = mybir.AxisListType

N_CORES = 8
BS = 64
C_IN = 256
J = 10
D = 32
O = J * D          # 320
I = 1024           # 32*32 pixels
ROUTE_NUM = 3
B = BS // N_CORES  # 8 batches per core
N_IT = I // 128    # 8
N_KT = C_IN // 128 # 2
N_OT = 3           # o tiles: 128, 128, 64


def r(ap):
    return ap.bitcast(F32R)


def build_kernel():
    nc = bacc.Bacc("TRN2", target_bir_lowering=False, debug=False, num_devices=1)

    x_d = nc.dram_tensor("x", [B, C_IN, I], F32R, kind="ExternalInput")
    wt_d = nc.dram_tensor("wt", [C_IN, O], F32R, kind="ExternalInput")   # W.T
    wb_d = nc.dram_tensor("wb", [1, O], F32R, kind="ExternalInput")
    out_d = nc.dram_tensor("v", [B, J, D], F32, kind="ExternalOutput")

    ident_d = nc.inline_tensor(np.eye(16, dtype=np.float32), name="ident")
    bm = np.zeros((16, O), dtype=np.float32)
    for j in range(J):
        bm[j, D * j:D * j + D] = 1.0
    bmask_d = nc.inline_tensor(bm, name="bmask")
    c0_d = nc.inline_tensor(np.full((128, J), 1.0 / J, dtype=np.float32), name="c0")

    with tile.TileContext(nc) as tc:
        with ExitStack() as ctx:
            consts = ctx.enter_context(tc.tile_pool(name="consts", bufs=1))
            xpool = ctx.enter_context(tc.tile_pool(name="xp", bufs=4))
            ppT = ctx.enter_context(tc.tile_pool(name="ppT", bufs=B))
            ppO = ctx.enter_context(tc.tile_pool(name="ppO", bufs=B))
            st = ctx.enter_context(tc.tile_pool(name="st", bufs=6))
            cpool = ctx.enter_context(tc.tile_pool(name="cp", bufs=4))
            vpool = ctx.enter_context(tc.tile_pool(name="vp", bufs=4))


            # ---- constants (wt first: it gates the first conv matmul) ----
            wt_sb = consts.tile([128, N_KT * O], F32R)
            nc.sync.dma_start(
                out=wt_sb.rearrange("p (k o) -> p k o", o=O),
                in_=wt_d.ap().rearrange("(k p) o -> p k o", p=128),
            )
            C = {}
            globals_ps = {}

            def load_consts():
                # Emitted after batch 0's x DMA so these small transfers don't
                # delay the startup-critical x.
                bias_b128 = consts.tile([128, O], F32)
                wb_bc = bass.AP(
                    tensor=wb_d, offset=0, ap=[[0, 128], [1, O]]
                ).bitcast(F32)
                nc.sync.dma_start(out=bias_b128, in_=wb_bc)
                bias_col = consts.tile([128, N_OT], F32)
                for m in range(N_OT):
                    mw = 128 if m < 2 else 64
                    nc.sync.dma_start(
                        out=bias_col[0:mw, m:m + 1],
                        in_=wb_d.ap().bitcast(F32)[0:1, 128 * m:128 * m + mw],
                    )
                ident_sb = consts.tile([128, 16], F32)
                nc.sync.dma_start(out=ident_sb[:16], in_=ident_d.ap())
                bmask_sb = consts.tile([128, O], F32)
                nc.sync.dma_start(out=bmask_sb[:16], in_=bmask_d.ap())
                c0_sb = consts.tile([128, J], F32R)
                nc.sync.dma_start(out=c0_sb, in_=r(c0_d.ap()))
                ones_sb = consts.tile([128, N_IT], F32)
                nc.gpsimd.memset(ones_sb, 1.0)
                b_sb = consts.tile([128, B * N_IT * J], F32)
                vout = consts.tile([128, B * D], F32)
                C.update(
                    bias_b128=bias_b128, bias_col=bias_col, ident_sb=ident_sb,
                    bmask_sb=bmask_sb, c0_sb=c0_sb, b_sb=b_sb, vout=vout, ones=ones_sb,
                )

            predT = [None] * B
            pred = [None] * B

            def bslice(b):
                off = b * N_IT * J
                return C["b_sb"][:, off:off + N_IT * J]

            def conv_unit(b):
                x_sb = xpool.tile([128, N_KT * I], F32R, tag="x")
                chunks = [(0, 128), (128, 512), (512, 1024)] if b == 0 else [(0, 512), (512, 1024)]
                for lo, hi in chunks:
                    for k in range(N_KT):
                        nc.sync.dma_start(
                            out=x_sb[:, k * I + lo:k * I + hi],
                            in_=x_d.ap()[b][k * 128:(k + 1) * 128, lo:hi],
                        )
                if b == 0:
                    load_consts()
                pT = ppT.tile([128, N_IT * O], F32R, tag="predT")
                pr = ppO.tile([128, N_OT * I], BF16, tag="pred")
                # Interleave the two layouts so the DVE (predT) and ACT (pred)
                # evictions run concurrently instead of in phases.
                jobs = []
                for u in range(N_IT):
                    jobs.append(("T", u))
                    if u < 2 * N_OT:
                        jobs.append(("O", u))
                for kind, u in jobs:
                    ps = globals_ps["psc"].tile([128, 512], F32, tag="cv")
                    if kind == "T":
                        for k in range(N_KT):
                            nc.tensor.matmul(
                                ps[:, :O],
                                r(x_sb[:, k * I + u * 128:k * I + u * 128 + 128]),
                                wt_sb[:, k * O:(k + 1) * O],
                                start=(k == 0),
                                stop=(k == N_KT - 1),
                            )
                        # eviction fused with the conv-bias add
                        nc.vector.tensor_tensor(
                            pT[:, u * O:(u + 1) * O], ps[:, :O], C["bias_b128"], OP.add
                        )
                    else:
                        m, h = divmod(u, 2)
                        mw = 128 if m < 2 else 64
                        for k in range(N_KT):
                            nc.tensor.matmul(
                                ps[:mw],
                                wt_sb[:, k * O + m * 128:k * O + m * 128 + mw],
                                r(x_sb[:, k * I + h * 512:k * I + (h + 1) * 512]),
                                start=(k == 0),
                                stop=(k == N_KT - 1),
                            )
                        nc.scalar.activation(
                            pr[:mw, m * I + h * 512:m * I + (h + 1) * 512],
                            ps[:mw], AF.Identity,
                            bias=C["bias_col"][0:mw, m:m + 1], scale=1.0,
                        )
                predT[b] = pT
                pred[b] = pr

            def front_unit(b, it):
                last = it == ROUTE_NUM - 1
                # ---- softmax over j (free-dim groups of 10) ----
                if it > 0:
                    e_sb = st.tile([128, N_IT * J], F32, tag="e")
                    nc.scalar.activation(e_sb, bslice(b), AF.Exp)
                    den = st.tile([128, N_IT], F32, tag="den")
                    nc.vector.reduce_sum(
                        den, e_sb.rearrange("p (g j) -> p g j", j=J), axis=AX.X
                    )
                    rden = st.tile([128, N_IT], F32, tag="rden")
                    nc.gpsimd.tensor_tensor(rden, C["ones"], den, OP.divide)
                    c_sb = cpool.tile([128, N_IT * J], F32R, tag="c")
                    nc.vector.tensor_tensor(
                        c_sb.rearrange("p (g j) -> p g j", j=J),
                        e_sb.rearrange("p (g j) -> p g j", j=J),
                        rden.broadcast_to([128, N_IT, J]),
                        OP.mult,
                    )
                # ---- s = c . predT (contraction over i) ----
                ps_s = globals_ps["pss"].tile([128, O], F32, tag="s")
                for t in range(N_IT):
                    lhs = C["c0_sb"] if it == 0 else c_sb[:, t * J:(t + 1) * J]
                    nc.tensor.matmul(
                        ps_s[:J], lhs, predT[b][:, t * O:(t + 1) * O],
                        start=(t == 0), stop=(t == N_IT - 1),
                    )
                return ps_s

            def back_unit(b, it, ps_s):
                last = it == ROUTE_NUM - 1
                # ---- squash ----
                s_m = st.tile([128, O], F32, tag="s_m")
                nc.vector.tensor_tensor(s_m[:J], ps_s[:J], C["bmask_sb"][:J], OP.mult)
                ns = st.tile([128, 1], F32, tag="ns")
                if not last:
                    sq = st.tile([128, O], F32, tag="sq")
                    nc.scalar.activation(sq[:J], s_m[:J], AF.Square, accum_out=ns[:J])
                else:
                    s_cmp = st.tile([128, D], F32, tag="s_cmp")
                    nc.vector.reduce_sum(
                        s_cmp[:J],
                        s_m[:J].rearrange("p (j d) -> p d j", j=J),
                        axis=AX.X,
                    )
                    sq = st.tile([128, D], F32, tag="sqc")
                    nc.scalar.activation(sq[:J], s_cmp[:J], AF.Square, accum_out=ns[:J])
                # sqrt(ns) = exp(0.5*ln(ns)) — stays in the pinned ACT table
                lns = st.tile([128, 1], F32, tag="lns")
                nc.scalar.activation(lns[:J], ns[:J], AF.Ln)
                rt = st.tile([128, 1], F32, tag="rt")
                nc.scalar.activation(rt[:J], lns[:J], AF.Exp, scale=0.5)
                ns1 = st.tile([128, 1], F32, tag="ns1")
                nc.gpsimd.tensor_scalar_add(ns1[:J], ns[:J], 1.0)

                if last:
                    # vout = (s_cmp * sqrt(ns)) / (1 + ns) fused in one op
                    nc.vector.scalar_tensor_tensor(
                        C["vout"][:J, b * D:(b + 1) * D],
                        s_cmp[:J], rt[:J], ns1[:J].broadcast_to([J, D]),
                        OP.mult, OP.divide,
                    )
                    return
                rns1 = st.tile([128, 1], F32, tag="rns1")
                nc.vector.reciprocal(rns1[:J], ns1[:J])
                coeff = st.tile([128, 1], F32, tag="coeff")
                nc.gpsimd.tensor_tensor(coeff[:J], rt[:J], rns1[:J], OP.mult)

                v_full = st.tile([128, O], F32, tag="v_full")
                nc.gpsimd.tensor_scalar_mul(v_full[:J], s_m[:J], coeff[:J])

                # V: transpose v into block-diagonal [o-part, j]
                ps_tv = globals_ps["pst"].tile([128, 32], F32, tag="tv")
                for k in range(N_OT):
                    kw = 128 if k < 2 else 64
                    nc.tensor.transpose(
                        ps_tv[:kw, k * J:(k + 1) * J],
                        v_full[:J, k * 128:k * 128 + kw],
                        C["ident_sb"][:J, :J],
                    )
                vb = vpool.tile([128, 32], BF16, tag="vb")
                nc.scalar.activation(vb[:, :2 * J], ps_tv[:, :2 * J], AF.Identity, scale=1.0)
                nc.vector.tensor_copy(vb[:64, 2 * J:3 * J], ps_tv[:64, 2 * J:3 * J])

                # delta^T: [i-part, j] tiles straight from PE, no transposes
                d_ps = globals_ps["psb"].tile([128, N_IT * J], F32, tag="d")
                for t in range(N_IT):
                    for k in range(N_OT):
                        kw = 128 if k < 2 else 64
                        nc.tensor.matmul(
                            d_ps[:, t * J:(t + 1) * J],
                            pred[b][:kw, k * I + t * 128:k * I + t * 128 + 128],
                            vb[:kw, k * J:(k + 1) * J],
                            start=(k == 0),
                            stop=(k == N_OT - 1),
                        )
                dst = bslice(b)
                if it == 0:
                    nc.vector.tensor_copy(dst, d_ps)
                else:
                    nc.vector.tensor_tensor(dst, d_ps, dst, OP.add)

            with tc.tile_pool(name="psc", bufs=4, space="PSUM") as ps_conv_:
                globals_ps["psc"] = ps_conv_
                for b in range(B):
                    conv_unit(b)
            psb = ctx.enter_context(tc.tile_pool(name="psb", bufs=2, space="PSUM"))
            pss = ctx.enter_context(tc.tile_pool(name="pss", bufs=4, space="PSUM"))
            pst = ctx.enter_context(tc.tile_pool(name="pst", bufs=2, space="PSUM"))
            globals_ps["psb"], globals_ps["pss"], globals_ps["pst"] = psb, pss, pst

            PIPE = 6
            seq = [(b, it) for it in range(ROUTE_NUM) for b in range(B)]
            fr = {}
            for i in range(len(seq) + PIPE):
                if i < len(seq):
                    b, it = seq[i]
                    fr[(b, it)] = front_unit(b, it)
                j = i - PIPE
                if j >= 0:
                    b, it = seq[j]
                    back_unit(b, it, fr.pop((b, it)))
                    if it == ROUTE_NUM - 1 and b in (3, B - 1):
                        h = 0 if b == 3 else 1
                        nc.sync.dma_start(
                            out=out_d.ap()[h * 4:(h + 1) * 4].rearrange("b j d -> j b d"),
                            in_=C["vout"][:J, h * 4 * D:(h + 1) * 4 * D]
                                .rearrange("p (b d) -> p b d", d=D),
                        )

    nc.compile()
    return nc


_NC_CACHE = None
LAST_RESULT = None


def kernel(x: np.ndarray, W: np.ndarray, W_b: np.ndarray) -> np.ndarray:
    global _NC_CACHE
    if _NC_CACHE is None:
        _NC_CACHE = build_kernel()
    nc = _NC_CACHE

    x = np.ascontiguousarray(x.reshape(BS, C_IN, I), dtype=np.float32)
    wt = np.ascontiguousarray(W.T, dtype=np.float32)
    wb = np.ascontiguousarray(W_b.reshape(1, O), dtype=np.float32)

    in_maps = [
        {
            "x": np.ascontiguousarray(x[c * B:(c + 1) * B]),
            "wt": wt,
            "wb": wb,
        }
        for c in range(N_CORES)
    ]
    import os
    trace = bool(int(os.environ.get("KERNEL_TRACE", "0")))
    res = run_bass_kernel_spmd(
        nc, in_maps, core_ids=list(range(N_CORES)), trace=trace
    )
    if trace:
        global LAST_RESULT
        LAST_RESULT = res
    out = np.concatenate([res.results[c]["v"] for c in range(N_CORES)], axis=0)
    return out.astype(np.float32)


if __name__ == "__main__":
    rng = np.random.default_rng(0)
    x = rng.standard_normal((BS, C_IN, 32, 32), dtype=np.float32)
    W = (rng.standard_normal((O, C_IN)) * 0.02).astype(np.float32)
    W_b = (rng.standard_normal((O,)) * 0.02).astype(np.float32)
    v = kernel(x=x, W=W, W_b=W_b)
    print(v.shape, v.dtype, float(np.abs(v).max()))


# revision 19
# speedup vs baseline: 1.0457x; 1.0348x over previous
"""Trainium2 Bass kernel for CapLayer2 (1x1-conv capsule layer with dynamic routing).

Sharding: data-parallel over batch — 8 batches per core on 8 NeuronCores.

Per-core design (diagonal wavefront over 8 batches):
  - Emission interleaves conv(b), it0(b-1), it1(b-2), it2(b-3) so the
    routing chains (softmax -> s -> squash -> V -> delta) hide inside the
    PE-saturated conv stream and every engine queue stays deep.
  - The 1x1 conv produces BOTH pred layouts on TensorE:
      predT [i-part, o] f32r  (for the s matmuls, contraction over i=1024)
      pred  [o-part, i] bf16  (stationary operand of the delta matmuls)
    with predT/pred tiles interleaved so their DVE/ACT bias evictions run
    concurrently instead of in phases.
  - delta is computed TRANSPOSED: matmul(lhsT=pred[o,i-chunk], rhs=V[o,j])
    gives [i-part, j] tiles straight from PE (no [10,1024] delta streams,
    no delta transposes); a single [128,80] DVE op folds them into b.
  - s/squash per batch use [10, 320] tiles; norms via ACT Square+accum;
    sqrt as exp(0.5*ln) with the ACT table pinned so it never reloads.
  - The otherwise-idle GPSIMD (Pool) takes the squash scalar chain and the
    v_full scaling; the final v is reduced to compact [10, 32] per batch
    and written with one gathered DMA for all 8 batches.
"""

import numpy as np
from contextlib import ExitStack

import concourse.bacc as bacc
import concourse.bass as bass
import concourse.hw_specs as hw_specs

# Force every activation onto the one table that contains all functions this
# kernel uses (Copy/Identity/Exp/Ln/Square) so the ACT engine loads its
# function table exactly once instead of thrashing between sets.
_ONE_TABLE = "natural_log_exp_and_others"
_orig_get_tables = hw_specs.get_activation_tables


def _pinned_tables(arch):
    tabs = _orig_get_tables(arch)
    return {k: (v if k == _ONE_TABLE else set()) for k, v in tabs.items()}


bacc.get_activation_tables = _pinned_tables
import concourse.tile as tile
from concourse import mybir
from concourse.bass_utils import run_bass_kernel_spmd

F32 = mybir.dt.float32
F32R = mybir.dt.float32r
BF16 = mybir.dt.bfloat16
AF = mybir.ActivationFunctionType
OP = mybir.AluOpType
AX = mybir.AxisListType

N_CORES = 8
BS = 64
C_IN = 256
J = 10
D = 32
O = J * D          # 320
I = 1024           # 32*32 pixels
ROUTE_NUM = 3
B = BS // N_CORES  # 8 batches per core
N_IT = I // 128    # 8
N_KT = C_IN // 128 # 2
N_OT = 3           # o tiles: 128, 128, 64


def r(ap):
    return ap.bitcast(F32R)


def build_kernel():
    nc = bacc.Bacc("TRN2", target_bir_lowering=False, debug=False, num_devices=1)

    x_d = nc.dram_tensor("x", [B, C_IN, I], F32R, kind="ExternalInput")
    wt_d = nc.dram_tensor("wt", [C_IN, O], F32R, kind="ExternalInput")   # W.T
    wb_d = nc.dram_tensor("wb", [1, O], F32R, kind="ExternalInput")
    out_d = nc.dram_tensor("v", [B, J, D], F32, kind="ExternalOutput")

    ident_d = nc.inline_tensor(np.eye(16, dtype=np.float32), name="ident")
    bm = np.zeros((16, O), dtype=np.float32)
    for j in range(J):
        bm[j, D * j:D * j + D] = 1.0
    bmask_d = nc.inline_tensor(bm, name="bmask")
    c0_d = nc.inline_tensor(np.full((128, J), 1.0 / J, dtype=np.float32), name="c0")

    with tile.TileContext(nc) as tc:
        with ExitStack() as ctx:
            consts = ctx.enter_context(tc.tile_pool(name="consts", bufs=1))
            xpool = ctx.enter_context(tc.tile_pool(name="xp", bufs=3))
            ppT = ctx.enter_context(tc.tile_pool(name="ppT", bufs=B))
            ppO = ctx.enter_context(tc.tile_pool(name="ppO", bufs=B))
            st = ctx.enter_context(tc.tile_pool(name="st", bufs=6))
            fpool = ctx.enter_context(tc.tile_pool(name="fp", bufs=8))
            cpool = ctx.enter_context(tc.tile_pool(name="cp", bufs=8))
            vpool = ctx.enter_context(tc.tile_pool(name="vp", bufs=8))


            # ---- constants (wt first: it gates the first conv matmul) ----
            wt_sb = consts.tile([128, N_KT * O], F32R)
            for k in range(N_KT):
                nc.sync.dma_start(
                    out=wt_sb[:, k * O:(k + 1) * O],
                    in_=wt_d.ap()[k * 128:(k + 1) * 128, :],
                )
            C = {}
            globals_ps = {}

            def load_consts():
                # Emitted after batch 0's x DMA so these small transfers don't
                # delay the startup-critical x.
                bias_b128 = consts.tile([128, O], F32)
                wb_bc = bass.AP(
                    tensor=wb_d, offset=0, ap=[[0, 128], [1, O]]
                ).bitcast(F32)
                nc.sync.dma_start(out=bias_b128, in_=wb_bc)
                bias_col = consts.tile([128, N_OT], F32)
                for m in range(N_OT):
                    mw = 128 if m < 2 else 64
                    nc.sync.dma_start(
                        out=bias_col[0:mw, m:m + 1],
                        in_=wb_d.ap().bitcast(F32)[0:1, 128 * m:128 * m + mw],
                    )
                ident_sb = consts.tile([128, 16], F32)
                nc.sync.dma_start(out=ident_sb[:16], in_=ident_d.ap())
                bmask_sb = consts.tile([128, O], F32)
                nc.sync.dma_start(out=bmask_sb[:16], in_=bmask_d.ap())
                c0_sb = consts.tile([128, J], F32R)
                nc.sync.dma_start(out=c0_sb, in_=r(c0_d.ap()))
                ones_sb = consts.tile([128, N_IT], F32)
                nc.gpsimd.memset(ones_sb, 1.0)
                b_sb = consts.tile([128, B * N_IT * J], F32)
                vout = consts.tile([128, B * D], F32)
                C.update(
                    bias_b128=bias_b128, bias_col=bias_col, ident_sb=ident_sb,
                    bmask_sb=bmask_sb, c0_sb=c0_sb, b_sb=b_sb, vout=vout, ones=ones_sb,
                )

            predT = [None] * B
            pred = [None] * B

            def bslice(b):
                off = b * N_IT * J
                return C["b_sb"][:, off:off + N_IT * J]

            def conv_unit(b):
                x_sb = xpool.tile([128, N_KT * I], F32R, tag="x")
                chunks = [(0, 128), (128, 512), (512, 1024)] if b == 0 else [(0, 512), (512, 1024)]
                for lo, hi in chunks:
                    for k in range(N_KT):
                        nc.sync.dma_start(
                            out=x_sb[:, k * I + lo:k * I + hi],
                            in_=x_d.ap()[b][k * 128:(k + 1) * 128, lo:hi],
                        )
                if b == 0:
                    load_consts()
                pT = ppT.tile([128, N_IT * O], F32R, tag="predT")
                pr = ppO.tile([128, N_OT * I], BF16, tag="pred")
                # Interleave the two layouts so the DVE (predT) and ACT (pred)
                # evictions run concurrently instead of in phases.
                jobs = []
                for u in range(N_IT):
                    jobs.append(("T", u))
                    if u < 2 * N_OT:
                        jobs.append(("O", u))
                for kind, u in jobs:
                    ps = globals_ps["psc"].tile([128, 512], F32, tag="cv")
                    if kind == "T":
                        for k in range(N_KT):
                            nc.tensor.matmul(
                                ps[:, :O],
                                r(x_sb[:, k * I + u * 128:k * I + u * 128 + 128]),
                                wt_sb[:, k * O:(k + 1) * O],
                                start=(k == 0),
                                stop=(k == N_KT - 1),
                            )
                        # eviction fused with the conv-bias add
                        nc.vector.tensor_tensor(
                            pT[:, u * O:(u + 1) * O], ps[:, :O], C["bias_b128"], OP.add
                        )
                    else:
                        m, h = divmod(u, 2)
                        mw = 128 if m < 2 else 64
                        for k in range(N_KT):
                            nc.tensor.matmul(
                                ps[:mw],
                                wt_sb[:, k * O + m * 128:k * O + m * 128 + mw],
                                r(x_sb[:, k * I + h * 512:k * I + (h + 1) * 512]),
                                start=(k == 0),
                                stop=(k == N_KT - 1),
                            )
                        nc.scalar.activation(
                            pr[:mw, m * I + h * 512:m * I + (h + 1) * 512],
                            ps[:mw], AF.Identity,
                            bias=C["bias_col"][0:mw, m:m + 1], scale=1.0,
                        )
                predT[b] = pT
                pred[b] = pr

            def front_unit(b, it):
                last = it == ROUTE_NUM - 1
                # ---- softmax over j (free-dim groups of 10) ----
                if it > 0:
                    e_sb = fpool.tile([128, N_IT * J], F32, tag="e")
                    nc.scalar.activation(e_sb, bslice(b), AF.Exp)
                    den = fpool.tile([128, N_IT], F32, tag="den")
                    nc.vector.reduce_sum(
                        den, e_sb.rearrange("p (g j) -> p g j", j=J), axis=AX.X
                    )
                    rden = fpool.tile([128, N_IT], F32, tag="rden")
                    nc.gpsimd.tensor_tensor(rden, C["ones"], den, OP.divide)
                    c_sb = cpool.tile([128, N_IT * J], F32R, tag="c")
                    nc.vector.tensor_tensor(
                        c_sb.rearrange("p (g j) -> p g j", j=J),
                        e_sb.rearrange("p (g j) -> p g j", j=J),
                        rden.broadcast_to([128, N_IT, J]),
                        OP.mult,
                    )
                # ---- s = c . predT (contraction over i) ----
                ps_s = globals_ps["pss"].tile([128, O], F32, tag="s")
                for t in range(N_IT):
                    lhs = C["c0_sb"] if it == 0 else c_sb[:, t * J:(t + 1) * J]
                    nc.tensor.matmul(
                        ps_s[:J], lhs, predT[b][:, t * O:(t + 1) * O],
                        start=(t == 0), stop=(t == N_IT - 1),
                    )
                return ps_s

            def back_unit(b, it, ps_s):
                last = it == ROUTE_NUM - 1
                # ---- squash ----
                s_m = st.tile([128, O], F32, tag="s_m")
                nc.vector.tensor_tensor(s_m[:J], ps_s[:J], C["bmask_sb"][:J], OP.mult)
                ns = st.tile([128, 1], F32, tag="ns")
                if not last:
                    sq = st.tile([128, O], F32, tag="sq")
                    nc.scalar.activation(sq[:J], s_m[:J], AF.Square, accum_out=ns[:J])
                else:
                    s_cmp = st.tile([128, D], F32, tag="s_cmp")
                    nc.vector.reduce_sum(
                        s_cmp[:J],
                        s_m[:J].rearrange("p (j d) -> p d j", j=J),
                        axis=AX.X,
                    )
                    sq = st.tile([128, D], F32, tag="sqc")
                    nc.scalar.activation(sq[:J], s_cmp[:J], AF.Square, accum_out=ns[:J])
                # sqrt(ns) = exp(0.5*ln(ns)) — stays in the pinned ACT table
                lns = st.tile([128, 1], F32, tag="lns")
                nc.scalar.activation(lns[:J], ns[:J], AF.Ln)
                rt = st.tile([128, 1], F32, tag="rt")
                nc.scalar.activation(rt[:J], lns[:J], AF.Exp, scale=0.5)
                ns1 = st.tile([128, 1], F32, tag="ns1")
                nc.gpsimd.tensor_scalar_add(ns1[:J], ns[:J], 1.0)

                if last:
                    # vout = (s_cmp * sqrt(ns)) / (1 + ns) fused in one op
                    nc.vector.scalar_tensor_tensor(
                        C["vout"][:J, b * D:(b + 1) * D],
                        s_cmp[:J], rt[:J], ns1[:J].broadcast_to([J, D]),
                        OP.mult, OP.divide,
                    )
                    return
                rns1 = st.tile([128, 1], F32, tag="rns1")
                nc.vector.reciprocal(rns1[:J], ns1[:J])
                coeff = st.tile([128, 1], F32, tag="coeff")
                nc.gpsimd.tensor_tensor(coeff[:J], rt[:J], rns1[:J], OP.mult)

                v_full = st.tile([128, O], F32, tag="v_full")
                nc.gpsimd.tensor_scalar_mul(v_full[:J], s_m[:J], coeff[:J])

                # V: transpose v into block-diagonal [o-part, j]
                ps_tv = globals_ps["pst"].tile([128, 32], F32, tag="tv")
                for k in range(N_OT):
                    kw = 128 if k < 2 else 64
                    nc.tensor.transpose(
                        ps_tv[:kw, k * J:(k + 1) * J],
                        v_full[:J, k * 128:k * 128 + kw],
                        C["ident_sb"][:J, :J],
                    )
                vb = vpool.tile([128, 32], BF16, tag="vb")
                nc.scalar.activation(vb[:, :2 * J], ps_tv[:, :2 * J], AF.Identity, scale=1.0)
                nc.vector.tensor_copy(vb[:64, 2 * J:3 * J], ps_tv[:64, 2 * J:3 * J])

                # delta^T: [i-part, j] tiles straight from PE, no transposes
                d_ps = globals_ps["psb"].tile([128, N_IT * J], F32, tag="d")
                for t in range(N_IT):
                    for k in range(N_OT):
                        kw = 128 if k < 2 else 64
                        nc.tensor.matmul(
                            d_ps[:, t * J:(t + 1) * J],
                            pred[b][:kw, k * I + t * 128:k * I + t * 128 + 128],
                            vb[:kw, k * J:(k + 1) * J],
                            start=(k == 0),
                            stop=(k == N_OT - 1),
                        )
                dst = bslice(b)
                if it == 0:
                    nc.vector.tensor_copy(dst, d_ps)
                else:
                    nc.vector.tensor_tensor(dst, d_ps, dst, OP.add)

            with tc.tile_pool(name="psc", bufs=4, space="PSUM") as ps_conv_:
                globals_ps["psc"] = ps_conv_
                for b in range(B):
                    conv_unit(b)
            psb = ctx.enter_context(tc.tile_pool(name="psb", bufs=2, space="PSUM"))
            pss = ctx.enter_context(tc.tile_pool(name="pss", bufs=4, space="PSUM"))
            pst = ctx.enter_context(tc.tile_pool(name="pst", bufs=2, space="PSUM"))
            globals_ps["psb"], globals_ps["pss"], globals_ps["pst"] = psb, pss, pst

            PIPE = 8
            seq = [(b, it) for it in range(ROUTE_NUM) for b in range(B)]
            fr = {}
            for i in range(len(seq) + PIPE):
                if i < len(seq):
                    b, it = seq[i]
                    fr[(b, it)] = front_unit(b, it)
                j = i - PIPE
                if j >= 0:
                    b, it = seq[j]
                    back_unit(b, it, fr.pop((b, it)))
                    if it == ROUTE_NUM - 1 and b in (3, B - 1):
                        h = 0 if b == 3 else 1
                        nc.sync.dma_start(
                            out=out_d.ap()[h * 4:(h + 1) * 4].rearrange("b j d -> j b d"),
                            in_=C["vout"][:J, h * 4 * D:(h + 1) * 4 * D]
                                .rearrange("p (b d) -> p b d", d=D),
                        )

    nc.compile()
    return nc


_NC_CACHE = None
LAST_RESULT = None


def kernel(x: np.ndarray, W: np.ndarray, W_b: np.ndarray) -> np.ndarray:
    global _NC_CACHE
    if _NC_CACHE is None:
        _NC_CACHE = build_kernel()
    nc = _NC_CACHE

    x = np.ascontiguousarray(x.reshape(BS, C_IN, I), dtype=np.float32)
    wt = np.ascontiguousarray(W.T, dtype=np.float32)
    wb = np.ascontiguousarray(W_b.reshape(1, O), dtype=np.float32)

    in_maps = [
        {
            "x": np.ascontiguousarray(x[c * B:(c + 1) * B]),
            "wt": wt,
            "wb": wb,
        }
        for c in range(N_CORES)
    ]
    import os
    trace = bool(int(os.environ.get("KERNEL_TRACE", "0")))
    res = run_bass_kernel_spmd(
        nc, in_maps, core_ids=list(range(N_CORES)), trace=trace
    )
    if trace:
        global LAST_RESULT
        LAST_RESULT = res
    out = np.concatenate([res.results[c]["v"] for c in range(N_CORES)], axis=0)
    return out.astype(np.float32)


if __name__ == "__main__":
    rng = np.random.default_rng(0)
    x = rng.standard_normal((BS, C_IN, 32, 32), dtype=np.float32)
    W = (rng.standard_normal((O, C_IN)) * 0.02).astype(np.float32)
    W_b = (rng.standard_normal((O,)) * 0.02).astype(np.float32)
    v = kernel(x=x, W=W, W_b=W_b)
    print(v.shape, v.dtype, float(np.abs(v).max()))
